# revision 20
# baseline (speedup 1.0000x reference)
"""Trainium2 Bass kernel for nn_Attention_23424751632639.

Computation (per (b,h)):  out = tril_strict(rope(Q) @ rope(Q).T / sqrt(N)) @ V
Chunked linear attention (exact reordering of the sums):
  out_c = QR_c @ M_c  +  strict_mask(QR_c @ QR_c^T) @ V_c
  M_{c+1} = M_c + QR_c^T @ V_c          (M is the [64,64] running state)
with QR = rope(Q) * N**-0.25 (scale folded into the cos/sin tables).

v4 design:
  * Host marshals Q, swap(Q), V to bf16 in chunk-major j-major layout
    [128, (j, c, n)] (two heads packed per tensor) -> contiguous DMA
    lines; output returned in bf16 (tolerance 2e-2).
  * RoPE on device in bf16: DVE mul + GpSimd mul + DVE add, staged.
  * QR^T strips per head via batched DMA-xbar blocked transposes
    ([128, s*64] -> [64, s, 128]), one per (pair, head, stage), split
    across the sync/scalar HWDGE rings.  All matmul operands stay at
    partition base 0: consecutive matmuls whose operands sit at
    different partition bases (0 then 64) crash the device.
  * Per-pair PSUM tags so the two head-pairs pipeline independently.
  * Mask: even chunks DVE scalar_tensor_tensor straight from PSUM;
    odd chunks ACT copy + DVE bf16 multiply.
  * State for both heads accumulates into one [64,128] PSUM tile
    (column halves) -> single snapshot copy per chunk (DVE/ACT
    alternating) feeds both heads' inter matmuls.
  * Outputs accumulate 4 chunks per PSUM bank; ACT evac + one DMA per
    512 columns.

Sharding: B*H = 32 (b,h) pairs -> 4 per core across 8 cores; no collectives.
"""

import math
import sys

import numpy as np

if "/opt/trn_rl_repo" not in sys.path:
    sys.path.insert(0, "/opt/trn_rl_repo")

import ml_dtypes

BF16 = ml_dtypes.bfloat16

B, H, T, N = 2, 16, 4096, 64
THETA = 2.0 ** 16
NCORES = 8
HPC = (B * H) // NCORES      # heads per core (4)
NPAIR = HPC // 2             # head pairs per core (2)
CH = T // 128                # 128-row chunks per head (32)


def _host_tables(t_len=T):
    """Scaled RoPE tables, chunk-major j-major duplicated: [128, 2*ch*N]."""
    ch = t_len // 128
    n = np.arange(N, dtype=np.float64)
    tq = np.floor(n / 2.0) * 2.0
    freqs = 1.0 / (THETA ** (tq / N)) / (2.0 * math.pi)
    t = np.arange(t_len, dtype=np.float64)[:, None]
    ang = ((t * freqs[None, :]) % 1.0) * (2.0 * math.pi)
    sc = float(N) ** -0.25
    cc = (np.cos(ang) * sc).astype(np.float32)
    ss = (np.sin(ang) * sc).astype(np.float32)
    ss[:, 0::2] *= -1.0

    def pack(x):  # [t, N] -> [128, (j=2, ch, N)]
        xc = x.reshape(ch, 128, N).transpose(1, 0, 2)       # [128, ch, N]
        x2 = np.stack([xc, xc], axis=1)                     # [128, 2, ch, N]
        return np.ascontiguousarray(x2.reshape(128, 2 * ch * N).astype(BF16))

    return pack(cc), pack(ss)


def _mask():
    # [key-part, query-free]: keep scores where key < query (strict causal)
    m = np.triu(np.ones((128, 128), dtype=np.float32), k=1)
    return np.ascontiguousarray(np.concatenate([m, m], axis=1).astype(BF16))


def _pack_pair(x, t_len=T):  # x [2, t, N] -> [128, (j, c, n)] bf16
    ch = t_len // 128
    xc = x.reshape(2, ch, 128, N).transpose(2, 0, 1, 3)     # [128, 2, ch, N]
    return np.ascontiguousarray(xc.reshape(128, 2 * ch * N).astype(BF16))


def _stages(ch=CH):
    # stage*64 must be a multiple of 128 (xbar tile cols) -> even stages
    if ch <= 8:
        return [min(2, ch)] + ([ch - 2] if ch > 2 else [])
    out = [2, 2, 4]
    left = ch - 8
    while left > 0:
        out.append(min(8, left))
        left -= 8
    return out


def build_program(t_len=T, debug_stop=None):
    import concourse.mybir as mybir
    import concourse.tile as tile
    from concourse import bacc

    f32 = mybir.dt.float32
    bf = mybir.dt.bfloat16
    ch = t_len // 128
    W = ch * 128   # free width of a packed pair tensor (j, c, n)
    W2 = ch * 64   # one head's width in (j, c, n) layout
    WT = ch * 128  # width of a transposed strip tile

    nc = bacc.Bacc(None, target_bir_lowering=False)
    q = nc.dram_tensor("q", [NPAIR, 128, W], bf, kind="ExternalInput")
    qs = nc.dram_tensor("qs", [NPAIR, 128, W], bf, kind="ExternalInput")
    v = nc.dram_tensor("v", [NPAIR, 128, W], bf, kind="ExternalInput")
    cc = nc.dram_tensor("cc", [128, W], bf, kind="ExternalInput")
    ss = nc.dram_tensor("ss", [128, W], bf, kind="ExternalInput")
    mu = nc.dram_tensor("mu", [128, 256], bf, kind="ExternalInput")
    o = nc.dram_tensor("o", [NPAIR, 128, WT], bf, kind="ExternalOutput")

    with tile.TileContext(nc) as tc:
        with (
            tc.tile_pool(name="const", bufs=1) as constp,
            tc.tile_pool(name="pair", bufs=1) as pairp,
            tc.tile_pool(name="rope", bufs=3) as ropep,
            tc.tile_pool(name="work", bufs=2) as workp,
            tc.tile_pool(name="ost", bufs=2) as ostp,
            tc.tile_pool(name="ps", bufs=2, space="PSUM") as psp,
            tc.tile_pool(name="pso", bufs=1, space="PSUM") as psop,
            tc.tile_pool(name="psm", bufs=1, space="PSUM") as psmp,
        ):
            cc_sb = constp.tile([128, W], bf)
            ss_sb = constp.tile([128, W], bf)
            mu_sb = constp.tile([128, 256], bf)
            nc.sync.dma_start(mu_sb[:], mu[:])
            # table loads split so the first rope stage starts early
            wq = W // 4
            for k in range(4):
                tl = slice(k * wq, (k + 1) * wq)
                nc.sync.dma_start(cc_sb[:, tl], cc[:, tl])
                nc.sync.dma_start(ss_sb[:, tl], ss[:, tl])

            qr2 = {}
            qrt = {}
            v_sb = {}
            for g in range(NPAIR):
                qr2[g] = pairp.tile([128, W], bf, name=f"qr{g}", tag=f"qr{g}")
                v_sb[g] = pairp.tile([128, W], bf, name=f"v{g}", tag=f"v{g}")
                for j in range(2):
                    qrt[g, j] = pairp.tile([64, WT], bf, name=f"qrt{g}{j}",
                                           tag=f"qrt{g}{j}")

            # ---- RoPE + per-stage blocked transposes, both pairs ----
            cbase = 0
            for stage in _stages(ch):
                for g in range(NPAIR):
                    for j in range(2):
                        fsl = slice(j * W2 + cbase * 64,
                                    j * W2 + (cbase + stage) * 64)
                        fw = stage * 64
                        jw = 8 * 64
                        q_st = ropep.tile([128, 2 * jw], bf, name="qst",
                                          tag=f"q{g}")[:, j * jw:j * jw + fw]
                        qs_st = ropep.tile([128, 2 * jw], bf, name="qsst",
                                           tag=f"qs{g}")[:, j * jw:j * jw + fw]
                        nc.sync.dma_start(q_st, q[g][:, fsl])
                        nc.sync.dma_start(qs_st, qs[g][:, fsl])
                        nc.sync.dma_start(v_sb[g][:, fsl], v[g][:, fsl])
                        t1 = ropep.tile([128, 2 * jw], bf, name="t1",
                                        tag=f"t1{g}")[:, j * jw:j * jw + fw]
                        t2 = ropep.tile([128, 2 * jw], bf, name="t2",
                                        tag=f"t2{g}")[:, j * jw:j * jw + fw]
                        nc.vector.tensor_mul(t1, q_st, cc_sb[:, fsl])
                        nc.gpsimd.tensor_mul(t2, qs_st, ss_sb[:, fsl])
                        nc.vector.tensor_add(qr2[g][:, fsl], t1, t2)

                        tsl = slice(cbase * 128, (cbase + stage) * 128)
                        dst = qrt[g, j][:, tsl].rearrange(
                            "p (c t) -> p c t", c=stage)
                        if j == 0:
                            nc.sync.dma_start_transpose(dst, qr2[g][:, fsl])
                        else:
                            nc.scalar.dma_start_transpose(dst, qr2[g][:, fsl])
                cbase += stage

            # ---- main loop: both pairs interleaved chunk by chunk ----
            m_ps = {}
            mb_prev = {}
            ob = {}
            ost = {}
            for g in range(NPAIR):
                m_ps[g] = psmp.tile([64, 128], f32, name=f"m{g}",
                                    tag=f"m{g}", bufs=1)

            for c in range(ch):
                for g in range(NPAIR):
                    csl = slice(c * 128, (c + 1) * 128)
                    # intra scores for both heads -> one PSUM bank
                    p2 = psp.tile([128, 256], f32, name="p2", tag=f"p{g}")
                    pmm = workp.tile([128, 256], bf, name="pmm",
                                     tag=f"pmm{g}")
                    for j in range(2):
                        strip = qrt[g, j][:, csl]
                        nc.tensor.matmul(
                            p2[:, j * 128:(j + 1) * 128], strip, strip,
                            start=True, stop=True,
                        )
                    if c % 2 == 0:
                        nc.vector.scalar_tensor_tensor(
                            pmm[:], p2[:], 1.0, mu_sb[:],
                            mybir.AluOpType.mult, mybir.AluOpType.mult)
                    else:
                        pm = workp.tile([128, 256], bf, name="pm",
                                        tag=f"pm{g}")
                        nc.scalar.copy(pm[:], p2[:])
                        nc.vector.tensor_mul(pmm[:], pm[:], mu_sb[:])

                    # output accumulator: 4 chunks per PSUM bank
                    k4 = c % 4
                    if k4 == 0:
                        ob[g] = psop.tile([128, 512], f32, name=f"ob{g}",
                                          tag=f"ob{g}")
                        ost[g] = ostp.tile([128, 512], bf, name=f"ostt{g}",
                                           tag=f"ost{g}")
                    osl = ob[g][:, k4 * 128:(k4 + 1) * 128]

                    for j in range(2):
                        jsl = slice(j * 64, (j + 1) * 64)
                        hsl = slice(j * W2 + c * 64, j * W2 + (c + 1) * 64)
                        strip = qrt[g, j][:, csl]
                        if c > 0:
                            nc.tensor.matmul(
                                osl[:, jsl], strip, mb_prev[g][:, jsl],
                                start=True, stop=False,
                            )
                        nc.tensor.matmul(
                            osl[:, jsl], pmm[:, j * 128:(j + 1) * 128],
                            v_sb[g][:, hsl], start=(c == 0), stop=True,
                        )
                        # state: M += QR_c^T @ V_c (head j -> col half j)
                        nc.tensor.matmul(
                            m_ps[g][:, jsl],
                            qr2[g][:, hsl], v_sb[g][:, hsl],
                            start=(c == 0 and j == 0), stop=(c == ch - 1),
                            skip_group_check=True,
                        )
                    if c < ch - 1:
                        mb = workp.tile([64, 128], bf, name=f"mb{g}",
                                        tag=f"mb{g}")
                        if c % 2 == 0:
                            nc.vector.tensor_copy(mb[:], m_ps[g][:])
                        else:
                            nc.scalar.copy(mb[:], m_ps[g][:])
                        mb_prev[g] = mb

                    if k4 == 3:
                        nc.scalar.copy(ost[g][:], ob[g][:])
                        c0 = (c // 4) * 4
                        nc.sync.dma_start(
                            o[g][:, c0 * 128:(c0 + 4) * 128], ost[g][:])

    nc.compile()
    return nc


_CACHE = {}


def _get_program():
    if "nc" not in _CACHE:
        _CACHE["nc"] = build_program()
    return _CACHE["nc"]


def _in_maps(Q, V):
    """Host marshaling: full fp32 inputs -> per-core bf16 input maps."""
    Q = np.asarray(Q, dtype=np.float32).reshape(NCORES, HPC, T, N)
    V = np.asarray(V, dtype=np.float32).reshape(NCORES, HPC, T, N)
    # swap feature pairs (sign lives in the ss table)
    Qsw = np.ascontiguousarray(
        Q.reshape(NCORES, HPC, T, N // 2, 2)[..., ::-1]
    ).reshape(NCORES, HPC, T, N)
    cc, ss = _host_tables()
    mu = _mask()
    maps = []
    for i in range(NCORES):
        qp = np.stack([_pack_pair(Q[i, 2 * g:2 * g + 2]) for g in range(NPAIR)])
        qsp = np.stack(
            [_pack_pair(Qsw[i, 2 * g:2 * g + 2]) for g in range(NPAIR)])
        vp = np.stack([_pack_pair(V[i, 2 * g:2 * g + 2]) for g in range(NPAIR)])
        maps.append({"q": qp, "qs": qsp, "v": vp, "cc": cc, "ss": ss,
                     "mu": mu})
    return maps


def _unpack_out(results):
    """Per-core bf16 'o' tensors [NPAIR,128,CH*128] -> [B,H,T,N] fp32."""
    outs = []
    for r in results:
        x = np.asarray(r["o"]).reshape(NPAIR, 128, CH, 2, N)
        x = x.transpose(0, 3, 2, 1, 4).reshape(HPC, T, N)
        outs.append(x)
    return np.stack(outs).reshape(B, H, T, N).astype(np.float32)


def kernel(Q, V):
    from concourse.bass_utils import run_bass_kernel_spmd

    nc = _get_program()
    in_maps = _in_maps(Q, V)
    res = run_bass_kernel_spmd(nc, in_maps, core_ids=list(range(NCORES)))
    return _unpack_out(res.results)
